# revision 2
# baseline (speedup 1.0000x reference)
"""DistanceTransformLoss on 8 Trainium2 NeuronCores (Bass/Tile).

loss = BCEWithLogits(predictions, targets).mean()
       + sqrt( sum(pen) / max(count(pen != 0), 1) ),
  pen = (sigmoid(pred) > 0.5) * grassfire_dist_H(targets)

Sharding: data-parallel over batch N (32 images -> 4 per core). Each core
reduces its shard to per-partition partial sums (softplus, p*t, penalty,
count); the host combines the 8 small [128, 128] accumulator tiles in f64.

Per (image, w-block) iteration on a core:
  - load p, t natural [128h, 1024(hb,w)] chunks (contiguous 512B rows)
  - PE-transpose both into PSUM [128w, 1024h]
  - ACT: e = exp(p_T); softplus sum via ln(e + 1) accум; init = 1024 - 1024*t_T
  - DVE: grassfire = fwd min-scan then reversed bwd min-scan
    (native tensor_tensor_scan: state = min(state + 1, data1))
  - mask m = [e > 1] == [p > 0]; pen = m * d (GPSIMD); TS+accum sums/counts
  - TTR accumulates sum(p * t) in natural layout
"""
import sys

if "/opt/trn_rl_repo" not in sys.path:
    sys.path.insert(0, "/opt/trn_rl_repo")

import numpy as np
from contextlib import ExitStack

import concourse.bass as bass
import concourse.bacc as bacc
import concourse.tile as tile
from concourse import mybir, masks
from concourse.ap import AP
from concourse.bass_utils import run_bass_kernel_spmd

N_CORES = 8
N_PER_CORE = 4          # 32 images / 8 cores
H = 1024
W = 1024
WB = W // 128           # 8 w-blocks per image
HB = H // 128           # 8 h-blocks
N_ITERS = N_PER_CORE * WB   # 32 iterations per core

F32 = mybir.dt.float32
F16 = mybir.dt.float16
BF16 = mybir.dt.bfloat16

_CACHED_NC = None


def _rev_free(ap):
    """Reverse a 2-D [P, F] AP along the free dim."""
    (pstep, pcount), (fstep, fcount) = ap.ap[0], ap.ap[1]
    return AP(ap.tensor, ap.offset + (fcount - 1) * fstep,
              [[pstep, pcount], [-fstep, fcount]])


def _build_nc():
    nc = bacc.Bacc("TRN2", target_bir_lowering=False, debug=False,
                   enable_asserts=False)
    t_ext = nc.dram_tensor("targets", [N_PER_CORE, H, W], F32,
                           kind="ExternalInput").ap()
    p_ext = nc.dram_tensor("predictions", [N_PER_CORE, H, W], F32,
                           kind="ExternalInput").ap()
    acc_ext = nc.dram_tensor("acc", [128, 4 * N_ITERS], F32,
                             kind="ExternalOutput").ap()

    with tile.TileContext(nc) as tc, ExitStack() as ctx:
        const_pool = ctx.enter_context(tc.tile_pool(name="const", bufs=1))
        nat_pool = ctx.enter_context(tc.tile_pool(name="nat", bufs=3))
        tr_pool = ctx.enter_context(tc.tile_pool(name="tr", bufs=2))
        psum_pool = ctx.enter_context(tc.tile_pool(name="ps", bufs=2, space="PSUM"))
        acc_pool = ctx.enter_context(tc.tile_pool(name="acc", bufs=1))

        idn = const_pool.tile([128, 128], F32, tag="idn")
        masks.make_identity(nc, idn[:])
        ones = const_pool.tile([128, H], F16, tag="ones")
        nc.gpsimd.memset(ones[:], 1.0)

        accs = acc_pool.tile([128, 4 * N_ITERS], F32)
        nc.vector.memset(accs[:], 0.0)

        for it in range(N_ITERS):
            n, wb = divmod(it, WB)
            c_sp = accs[:, it:it + 1]
            c_pt = accs[:, N_ITERS + it:N_ITERS + it + 1]
            c_pen = accs[:, 2 * N_ITERS + it:2 * N_ITERS + it + 1]
            c_cnt = accs[:, 3 * N_ITERS + it:3 * N_ITERS + it + 1]

            t_nat = nat_pool.tile([128, H], F32, tag="t_nat")
            p_nat = nat_pool.tile([128, H], F32, tag="p_nat")
            nc.sync.dma_start(
                t_nat[:].rearrange("p (hb w) -> p hb w", hb=HB),
                t_ext[n, :, wb * 128:(wb + 1) * 128]
                .rearrange("(hb p) w -> p hb w", p=128),
            )
            nc.sync.dma_start(
                p_nat[:].rearrange("p (hb w) -> p hb w", hb=HB),
                p_ext[n, :, wb * 128:(wb + 1) * 128]
                .rearrange("(hb p) w -> p hb w", p=128),
            )

            psum_t = psum_pool.tile([128, H], F32, tag="psum_t")
            psum_p = psum_pool.tile([128, H], F32, tag="psum_p")
            for hb in range(HB):
                nc.tensor.transpose(
                    psum_t[:, hb * 128:(hb + 1) * 128],
                    t_nat[:, hb * 128:(hb + 1) * 128], idn[:])
                nc.tensor.transpose(
                    psum_p[:, hb * 128:(hb + 1) * 128],
                    p_nat[:, hb * 128:(hb + 1) * 128], idn[:])

            e_T = tr_pool.tile([128, H], F32, tag="e")
            sp_junk = tr_pool.tile([128, H], BF16, tag="spj")
            init_T = tr_pool.tile([128, H], F16, tag="init")
            nc.scalar.activation(e_T[:], psum_p[:],
                                 mybir.ActivationFunctionType.Exp)
            nc.scalar.activation(sp_junk[:], e_T[:],
                                 mybir.ActivationFunctionType.Ln,
                                 bias=1.0, accum_out=c_sp)
            nc.scalar.activation(init_T[:], psum_t[:],
                                 mybir.ActivationFunctionType.Copy,
                                 bias=1024.0, scale=-1024.0)

            fsc = tr_pool.tile([128, H], F16, tag="fsc")
            d_T = tr_pool.tile([128, H], F16, tag="d")
            nc.vector.tensor_tensor_scan(
                fsc[:], ones[:], init_T[:], 30000.0,
                mybir.AluOpType.add, mybir.AluOpType.min)
            nc.vector.tensor_tensor_scan(
                _rev_free(d_T[:]), ones[:], _rev_free(fsc[:]), 30000.0,
                mybir.AluOpType.add, mybir.AluOpType.min)

            m_T = tr_pool.tile([128, H], BF16, tag="m")
            pen = tr_pool.tile([128, H], F16, tag="pen")
            ind = tr_pool.tile([128, H], BF16, tag="ind")
            junk = tr_pool.tile([128, H], F16, tag="junk")
            junk2 = tr_pool.tile([128, H], BF16, tag="junk2")
            nc.vector.tensor_scalar(m_T[:], e_T[:], 1.0, None,
                                    mybir.AluOpType.is_gt)
            nc.gpsimd.tensor_tensor(pen[:], m_T[:], d_T[:],
                                    mybir.AluOpType.mult)
            nc.vector.tensor_scalar(junk[:], pen[:], 0.0, None,
                                    mybir.AluOpType.add, mybir.AluOpType.add,
                                    accum_out=c_pen)
            nc.vector.tensor_scalar(ind[:], pen[:], 0.0, None,
                                    mybir.AluOpType.is_gt, mybir.AluOpType.add,
                                    accum_out=c_cnt)
            nc.vector.scalar_tensor_tensor(
                junk2[:], p_nat[:], 0.0, t_nat[:],
                mybir.AluOpType.add, mybir.AluOpType.mult,
                accum_out=c_pt)

        nc.sync.dma_start(acc_ext, accs[:])

    nc.compile()
    return nc


def _get_nc():
    global _CACHED_NC
    if _CACHED_NC is None:
        _CACHED_NC = _build_nc()
    return _CACHED_NC


def _run(predictions, targets, trace=False, **trace_kwargs):
    """Run the SPMD kernel; returns (loss_scalar, BassKernelResults)."""
    p = np.ascontiguousarray(
        np.asarray(predictions, dtype=np.float32).reshape(32, H, W))
    t = np.ascontiguousarray(
        np.asarray(targets, dtype=np.float32).reshape(32, H, W))

    in_maps = []
    for c in range(N_CORES):
        sl = slice(c * N_PER_CORE, (c + 1) * N_PER_CORE)
        in_maps.append({
            "predictions": np.ascontiguousarray(p[sl]),
            "targets": np.ascontiguousarray(t[sl]),
        })

    nc = _get_nc()
    res = run_bass_kernel_spmd(nc, in_maps, list(range(N_CORES)),
                               trace=trace, **trace_kwargs)

    sum_sp = sum_pt = sum_pen = sum_cnt = 0.0
    for c in range(N_CORES):
        acc = np.asarray(res.results[c]["acc"], dtype=np.float64)
        sum_sp += acc[:, 0:N_ITERS].sum()
        sum_pt += acc[:, N_ITERS:2 * N_ITERS].sum()
        sum_pen += acc[:, 2 * N_ITERS:3 * N_ITERS].sum()
        sum_cnt += acc[:, 3 * N_ITERS:4 * N_ITERS].sum()

    n_elem = 32.0 * H * W
    bce = (sum_sp - sum_pt) / n_elem
    border = 0.0 if sum_pen == 0.0 else sum_pen / max(sum_cnt, 1.0)
    loss = bce + np.sqrt(border)
    return np.float32(loss), res


def kernel(predictions, targets):
    loss, _ = _run(predictions, targets)
    return np.asarray(loss, dtype=np.float32)
